# revision 1
# baseline (speedup 1.0000x reference)
# Trainium2 Bass kernel for the data-reuploading quantum-circuit model
# (nn_DARUAN_45311904972849), v2.
#
# Math: per (batch, dim) element a 2-state evolves through 8 reps of
# RZ(t0)*RY(t1)*RZ(w*x+b) plus a final RZ*RY, then <Z>. Tracked as a real
# Bloch vector (X, Y, Z):
#   RZ(t): (X, Y) <- (X cos t - Y sin t, X sin t + Y cos t)
#   RY(t): (X, Z) <- (X cos t + Z sin t, -X sin t + Z cos t)
# Per-dim RZ angles fold into the data angle biases on the host.
#
# Engine split (per rep):
#  - DVE: range-wrap custom op (angle pre-scaled by lambda so the cos
#    polynomial needs no extra constant), the 4 per-element RZ products,
#    and a deg-8 cos polynomial custom op on half the reps.
#  - ACT: table sin, abs+sin cos route on the other reps, and all
#    PSUM->SBUF casts (with the rep-1 constant-Z bias folded in).
#  - PE: every per-partition-scalar linear combination as diagonal
#    128x128 matmuls: the RY mixing (X' and Z chains), the rep-0 init
#    rotation, and the readout accumulation.
#  - Pool: the 2 RZ adds per rep via scalar_tensor_tensor and a couple
#    of Z casts.
# Xn feeds only PE and the next-rep products; Z never needs per-element
# compute outside PE (constant Z0 terms ride the cast biases).
#
# Sharding: dim axis split across 8 cores (256 dims each), full batch per
# core; x transposed on the host so SBUF tiles are (128 dims x batch).
import sys

sys.path.insert(0, '/opt/trn_rl_repo')
from contextlib import ExitStack

import numpy as np

import concourse.bass as bass  # noqa: F401
import concourse.tile as tile
from concourse import bacc, mybir
from concourse.bass_utils import run_bass_kernel_spmd

AFT = mybir.ActivationFunctionType
ALU = mybir.AluOpType
F32 = mybir.dt.float32
F16 = mybir.dt.float16

# ---- custom DVE ops ------------------------------------------------------
from concourse.dve_spec import Spec, Src0, C0, C1, C2, Zero, One  # noqa: E402
from concourse.dve_ops import DveOp, OPS  # noqa: E402


def _register_op(name, spec_body, ref):
    for op in OPS:
        if op.name == name:
            return op
    spec = Spec(body=spec_body, reference=ref)
    op = DveOp(name, spec, subdim=False, uops_sha={"v3": "", "v4": ""})
    OPS.append(op)
    import concourse.dve_ops as _dops
    _dops.CUSTOM_DVE_SPECS[op.name] = op.spec
    _dops._SUB_OPCODE_FOR_NAME[op.name] = (
        _dops._CUSTOM_DVE_ROW_BASE + len(OPS) - 1)
    assert _dops._SUB_OPCODE_FOR_NAME[op.name] < 0x20
    import re as _re
    for ver in ("v3", "v4"):
        try:
            op.compile(ver)
        except ValueError as e:
            m = _re.search(r'="([0-9a-f]{16})"', str(e))
            if not m:
                raise
            op.uops_sha[ver] = m.group(1)
            op.compile(ver)
    return op


def _wrap_affine_ref(in0, in1, s0, s1, imm2):
    y = in0 * s0 + s1
    d = y + y
    return y + imm2 * ((d < -imm2).astype(np.float32)
                       - (d > imm2).astype(np.float32))


# out = wrap(x*s0 + s1) into one period of +-imm2/2 (8 ALU stages).
_wy = Src0 * C0 + C1
_wd = _wy + _wy
WRAP_AFFINE = _register_op(
    "WRAP_AFFINE_DARUAN",
    _wy + C2 * ((_wd < (Zero - C2)) - (_wd > C2)),
    _wrap_affine_ref)

# deg-8 even cos polynomial on the lambda-scaled wrapped angle v:
# cos(v/lam) ~ 1 + v2*(C0 + v2*(C1 + v2*(C2 + v2))), v2 = v*v.
COS_LAM = 0.25706256873782884
COS_E0 = -7.564456024691932
COS_E1 = 9.510560454995842
COS_E2 = -4.65992802095955


def _cos8_ref(in0, in1, s0, s1, imm2):
    v2 = in0 * in0
    return 1.0 + v2 * (s0 + v2 * (s1 + v2 * (imm2 + v2)))


_v2 = Src0 * Src0
COS8 = _register_op(
    "COS8_DARUAN",
    (_v2 * (C0 + _v2 * (C1 + _v2 * (C2 + _v2)))) + One,
    _cos8_ref)

BATCH, DIM, REPS = 4096, 2048, 8
NCORES = 8
DPC = DIM // NCORES          # dims per core
PTILES = DPC // 128          # dim tiles per core
FCH = 1024                   # batch chunk (free dim; PSUM tile = 2 banks)
BCH = BATCH // FCH
PI = float(np.pi)

# reps whose cos runs on the DVE COS8 custom op (rest use ACT abs+sin)
DCOS = (0, 1, 3, 5, 7)

# param column layout (per dim)
_W = 0            # lam * w_r            cols 0..7
_BS = 8           # lam * folded bias    cols 8..15
_BX1 = 16         # st1 * nzp
_BZ1 = 17         # ct1 * nzp
_PB = 18          # output bias
_PI2 = 19         # pi/2 (ACT cos-route bias)
NPARAM = 20

# diagonal weight tile order (each [128, 128] f16):
#  0: nxp  1: -nyp  2: nyp                      (rep0: X0 = nxp*C0 - nyp*S0,
#                                                      Y0 = nyp*C0 + nxp*S0)
#  3: ct1  4: -st1                              (rep1 RY from X0s)
#  5 + 3*(r-2) + {0,1,2}: ct_r, st_r, -st_r     for r = 2..7 (18 tiles;
#                                                r=7 -st7 unused by Z but
#                                                kept for layout simplicity)
#  23: AX  24: -AZ*st7  25: AZ*ct7              (readout)
NW = 26


def _wslot(i):
    return slice(i * 128, (i + 1) * 128)


_CACHE = {}


def _build():
    nc = bacc.Bacc('TRN2', target_bir_lowering=False, debug=False,
                   num_devices=NCORES)
    xt_ext = nc.declare_dram_parameter("xt", [DPC, BATCH], F32, isOutput=False)
    pp_ext = nc.declare_dram_parameter("pp", [DPC, NPARAM], F32,
                                       isOutput=False)
    wt_ext = nc.declare_dram_parameter("wt", [DPC, NW * 128], F16,
                                       isOutput=False)
    yt_ext = nc.declare_dram_parameter("yt", [DPC, BATCH], F16, isOutput=True)

    inv_lam = 1.0 / COS_LAM

    with ExitStack() as ctx:
        tc = ctx.enter_context(tile.TileContext(nc))
        ppool = ctx.enter_context(tc.tile_pool(name="pp", bufs=2))
        wpool = ctx.enter_context(tc.tile_pool(name="wp", bufs=2))
        xpool = ctx.enter_context(tc.tile_pool(name="xp", bufs=5))
        apool = ctx.enter_context(tc.tile_pool(name="ang", bufs=5))
        tpool = ctx.enter_context(tc.tile_pool(name="trig", bufs=5))
        spool = ctx.enter_context(tc.tile_pool(name="state", bufs=7))
        mpool = ctx.enter_context(tc.tile_pool(name="mul", bufs=5))
        opool = ctx.enter_context(tc.tile_pool(name="out", bufs=4))
        pxp = ctx.enter_context(tc.tile_pool(name="pxp", bufs=2,
                                             space="PSUM"))
        pzp = ctx.enter_context(tc.tile_pool(name="pzp", bufs=2,
                                             space="PSUM"))

        def mm(out_t, w_ap, in_t, start, stop):
            # matmul accumulation groups must fit one PSUM bank (512 f32)
            for h in range(FCH // 512):
                sl = slice(h * 512, (h + 1) * 512)
                nc.tensor.matmul(out_t[:, sl], w_ap, in_t[:, sl],
                                 start=start, stop=stop)

        for dt in range(PTILES):
            pt = ppool.tile([128, NPARAM], F32, tag="pt")
            nc.sync.dma_start(pt[:], pp_ext[dt * 128:(dt + 1) * 128, :])
            wt = wpool.tile([128, NW * 128], F16, tag="wt")
            nc.sync.dma_start(wt[:], wt_ext[dt * 128:(dt + 1) * 128, :])

            def col(i):
                return pt[:, i:i + 1]

            def W(i):
                return wt[:, _wslot(i)]

            # software-pipeline all batch chunks of this dtile: emit rep r
            # for every chunk before rep r+1 so each in-order engine queue
            # always holds independent work from other chains.
            st = []
            for bc in range(BCH):
                xt = xpool.tile([128, FCH], F32, tag="x")
                nc.sync.dma_start(
                    xt[:], xt_ext[dt * 128:(dt + 1) * 128,
                                  bc * FCH:(bc + 1) * FCH])
                st.append({"xt": xt, "bc": bc})

            for r in range(REPS):
                for s in st:
                    xt = s["xt"]
                    # angle: ew = lam * wrap(w_r*x + bs_r), f32
                    ew = apool.tile([128, FCH], F32, tag="ew")
                    nc.vector._custom_dve(
                        WRAP_AFFINE, out=ew[:], in0=xt[:],
                        s0=col(_W + r), s1=col(_BS + r),
                        imm2=2 * PI * COS_LAM)
                    S = tpool.tile([128, FCH], F16, tag="S")
                    nc.scalar.activation(S[:], ew[:], AFT.Sin,
                                         bias=0.0, scale=inv_lam)
                    C = tpool.tile([128, FCH], F16, tag="C")
                    if r in DCOS:
                        nc.vector._custom_dve(
                            COS8, out=C[:], in0=ew[:],
                            s0=COS_E0, s1=COS_E1, imm2=COS_E2)
                    else:
                        UA = tpool.tile([128, FCH], F16, tag="UA")
                        nc.scalar.activation(UA[:], ew[:], AFT.Abs,
                                             bias=0.0, scale=inv_lam)
                        nc.scalar.activation(C[:], UA[:], AFT.Sin,
                                             bias=col(_PI2), scale=-1.0)

                    if r == 0:
                        X0p = pxp.tile([128, FCH], F32, tag="XP")
                        mm(X0p, W(0), C, True, False)
                        mm(X0p, W(1), S, False, True)
                        Y0p = pxp.tile([128, FCH], F32, tag="XP")
                        mm(Y0p, W(2), C, True, False)
                        mm(Y0p, W(0), S, False, True)
                        XN = spool.tile([128, FCH], F16, tag="XN")
                        nc.scalar.activation(XN[:], X0p[:], AFT.Identity,
                                             bias=0.0, scale=1.0)
                        YV = spool.tile([128, FCH], F16, tag="Y")
                        nc.scalar.activation(YV[:], Y0p[:], AFT.Identity,
                                             bias=0.0, scale=1.0)
                        s["XN"], s["Y"], s["ZS"] = XN, YV, None
                        continue

                    XN, YV, ZS = s["XN"], s["Y"], s["ZS"]
                    # ---- RY on PE: X'_r and Z_r diagonal matmuls --------
                    Xp = pxp.tile([128, FCH], F32, tag="XP")
                    if r == 1:
                        mm(Xp, W(3), XN, True, True)
                    else:
                        wct, wst, wmst = (5 + 3 * (r - 2),
                                          5 + 3 * (r - 2) + 1,
                                          5 + 3 * (r - 2) + 2)
                        mm(Xp, W(wct), XN, True, False)
                        mm(Xp, W(wst), ZS, False, True)
                    if r <= REPS - 2:
                        Zp = pzp.tile([128, FCH], F32, tag="ZP")
                        if r == 1:
                            mm(Zp, W(4), XN, True, True)
                        else:
                            mm(Zp, W(wmst), XN, True, False)
                            mm(Zp, W(wct), ZS, False, True)
                    else:
                        Zp = None

                    s["XN_prev"], s["ZS_prev"] = XN, ZS

                    # ---- casts PSUM -> SBUF f16 (bias folds const Z0) ---
                    XS = spool.tile([128, FCH], F16, tag="XS")
                    nc.scalar.activation(XS[:], Xp[:], AFT.Identity,
                                         bias=col(_BX1) if r == 1 else 0.0,
                                         scale=1.0)
                    if Zp is not None:
                        ZS = spool.tile([128, FCH], F16, tag="ZS")
                        nc.scalar.activation(ZS[:], Zp[:], AFT.Identity,
                                             bias=col(_BZ1) if r == 1 else 0.0,
                                             scale=1.0)
                    else:
                        ZS = None
                    s["ZS"] = ZS

                    # ---- RZ products (DVE) and adds (Pool) --------------
                    P1 = mpool.tile([128, FCH], F16, tag="P1")
                    nc.vector.tensor_mul(P1[:], C[:], XS[:])
                    P2 = mpool.tile([128, FCH], F16, tag="P2")
                    nc.vector.tensor_mul(P2[:], S[:], YV[:])
                    XNn = spool.tile([128, FCH], F16, tag="XN")
                    nc.gpsimd.tensor_sub(XNn[:], P1[:], P2[:])
                    if r <= REPS - 2:
                        P3 = mpool.tile([128, FCH], F16, tag="P3")
                        nc.vector.tensor_mul(P3[:], S[:], XS[:])
                        P4 = mpool.tile([128, FCH], F16, tag="P4")
                        nc.vector.tensor_mul(P4[:], C[:], YV[:])
                        YVn = spool.tile([128, FCH], F16, tag="Y")
                        nc.gpsimd.tensor_add(YVn[:], P3[:], P4[:])
                        s["Y"] = YVn
                    s["XN"] = XNn

            for s in st:
                # ---- readout: O = AX*Xn7 - AZ*st7*Xn6 + AZ*ct7*Zs6 + PB
                Op = pxp.tile([128, FCH], F32, tag="XP")
                mm(Op, W(23), s["XN"], True, False)
                mm(Op, W(24), s["XN_prev"], False, False)
                mm(Op, W(25), s["ZS_prev"], False, True)
                O = opool.tile([128, FCH], F16, tag="O")
                nc.scalar.activation(O[:], Op[:], AFT.Identity,
                                     bias=col(_PB), scale=1.0)
                bc = s["bc"]
                nc.sync.dma_start(
                    yt_ext[dt * 128:(dt + 1) * 128, bc * FCH:(bc + 1) * FCH],
                    O[:])

    nc.compile()
    return nc


def _fold_params(theta, pw, pb_, ow, ob):
    th = np.asarray(theta, np.float64)
    pw = np.asarray(pw, np.float64)
    pb_ = np.asarray(pb_, np.float64)
    ow = np.asarray(ow, np.float64)
    ob = np.asarray(ob, np.float64)
    t0 = th[:, :REPS, 0]
    t1 = th[:, :REPS, 1]
    tf0 = th[:, REPS, 0]
    tf1 = th[:, REPS, 1]

    P = np.zeros((DIM, NPARAM), np.float64)
    P[:, _W:_W + REPS] = pw * COS_LAM
    bs = pb_.copy()
    bs[:, :REPS - 1] += t0[:, 1:]
    bs[:, REPS - 1] += tf0
    P[:, _BS:_BS + REPS] = bs * COS_LAM

    ct = np.cos(t1)
    st = np.sin(t1)
    nxp = ct[:, 0] * np.cos(t0[:, 0])
    nyp = np.sin(t0[:, 0])
    nzp = -st[:, 0] * np.cos(t0[:, 0])
    P[:, _BX1] = st[:, 1] * nzp
    P[:, _BZ1] = ct[:, 1] * nzp
    P[:, _PB] = ob
    P[:, _PI2] = np.pi / 2
    AX = -ow * np.sin(tf1)
    AZ = ow * np.cos(tf1)

    # diagonal weight tiles, [DIM, NW*128] f16
    Wt = np.zeros((DIM, NW * 128), np.float32)

    def put(slot, vals):
        # vals: (DIM,) diagonal values; tile layout [k=128, m=128] per
        # 128-dim block: W[k, m] nonzero at k == m.
        for dtile in range(DIM // 128):
            sl = slice(dtile * 128, (dtile + 1) * 128)
            blk = Wt[sl, slot * 128:(slot + 1) * 128]
            np.fill_diagonal(blk, vals[sl])

    put(0, nxp)
    put(1, -nyp)
    put(2, nyp)
    put(3, ct[:, 1])
    put(4, -st[:, 1])
    for r in range(2, REPS):
        put(5 + 3 * (r - 2) + 0, ct[:, r])
        put(5 + 3 * (r - 2) + 1, st[:, r])
        put(5 + 3 * (r - 2) + 2, -st[:, r])
    put(23, AX)
    put(24, -AZ * st[:, REPS - 1])
    put(25, AZ * ct[:, REPS - 1])

    return P.astype(np.float32), Wt.astype(np.float16)


def _prep_in_maps(x, theta, preacts_weight, preacts_bias, postact_weights,
                  postact_bias):
    x = np.asarray(x, np.float32)
    P, Wt = _fold_params(theta, preacts_weight, preacts_bias,
                         postact_weights, postact_bias)
    in_maps = []
    for c in range(NCORES):
        sl = slice(c * DPC, (c + 1) * DPC)
        in_maps.append({
            "xt": np.ascontiguousarray(x[:, sl].T),
            "pp": np.ascontiguousarray(P[sl]),
            "wt": np.ascontiguousarray(Wt[sl]),
        })
    return in_maps


def _gather(results):
    out = np.empty((BATCH, DIM), np.float32)
    for c, r in enumerate(results):
        out[:, c * DPC:(c + 1) * DPC] = r["yt"].T.astype(np.float32)
    return out


def kernel(x, theta, preacts_weight, preacts_bias, postact_weights,
           postact_bias):
    if "nc" not in _CACHE:
        _CACHE["nc"] = _build()
    nc = _CACHE["nc"]
    in_maps = _prep_in_maps(x, theta, preacts_weight, preacts_bias,
                            postact_weights, postact_bias)
    try:
        res = run_bass_kernel_spmd(nc, in_maps, list(range(NCORES)))
    except Exception:
        # transient device errors usually clear on retry
        res = run_bass_kernel_spmd(nc, in_maps, list(range(NCORES)))
    return _gather(res.results)


def run_traced(inputs, trace_cores=None):
    """test harness helper: returns (out, exec_time_ns)."""
    if "nc" not in _CACHE:
        _CACHE["nc"] = _build()
    nc = _CACHE["nc"]
    in_maps = _prep_in_maps(**inputs)
    res = run_bass_kernel_spmd(nc, in_maps, list(range(NCORES)), trace=True,
                               trace_cores=trace_cores)
    return _gather(res.results), res.exec_time_ns

